# revision 35
# baseline (speedup 1.0000x reference)
"""BinASPP Trainium2 kernel (Bass/Tile), SPMD over 8 NeuronCores.

Strategy
--------
Data-parallel over batch: N=8 images -> 1 image per core.  binarize() forward
== sign(), so every conv is a matmul over {-1,0,+1} values: exact in fp8e4,
with exact integer accumulation in fp32 PSUM.  A dilated 3x3 conv is 9
shifted 1x1 convs (taps) over a zero-padded sign image resident in SBUF.

PE efficiency:
 - the padded sign image is stored k-interleaved [128, 2, pad_image] fp8 so a
   single DoubleRow matmul contracts all K=256 input channels at once;
 - moving-operand tiles are contiguous runs of 5 *padded* rows (N=440): a
   strided multi-dim rhs AP costs ~50 PE cycles per row break (measured
   381ns vs 213ns theoretical for [8x64] tiles), so we stream whole padded
   rows and discard the 24 pad columns when reading PSUM back.

Batch-norm is training-mode (batch statistics), so per-channel [sum, sumsq]
partials are AllReduced across the 8 cores.  One collective is issued per
(branch, out-channel-half), so each half's BN apply (s += a_c * clip(y, lo_c,
hi_c), lo/hi = (-/+1 - d_c)/a_c, d_c offsets deferred to a final += s0 pass)
runs on DVE underneath the remaining matmul stream; only the very last
half's allreduce+apply sits in the tail.  Branch sums ride free on the
PSUM->SBUF fp16 copy (ACT accum_out); sumsq is one Square pass per tile
(ACT, or DVE for the 1x1 branch whose PSUM drain is ACT-bound).  The stats
reductions stay off the in-order DVE queue (a stalled apply waiting on a
previous allreduce must not delay the next stats).  Branch outputs y (even
integers, |y| <= 2304) are staged in SBUF as fp16 (exact), so the apply pass
needs no recompute and no DRAM round trip.
"""

import numpy as np
import ml_dtypes
from contextlib import ExitStack

import concourse.bass as bass
import concourse.bacc as bacc
import concourse.mybir as mybir
import concourse.tile as tile
from concourse.bass_utils import run_bass_kernel_spmd

AF = mybir.ActivationFunctionType
ALU = mybir.AluOpType
AX = mybir.AxisListType
F32 = mybir.dt.float32
BF16 = mybir.dt.bfloat16
F16 = mybir.dt.float16
FP8 = mybir.dt.float8e4
DR = mybir.MatmulPerfMode.DoubleRow

P = 128
CIN = 256
COUT = 256
H = W = 64
HW = H * W
PAD = 12                      # max dilation rate
PH = PW = H + 2 * PAD         # 88
GUARD = 16                    # fp8 guard elements before/after each image
ILEN = GUARD + PH * PW + GUARD  # 7776 (multiple of 16 -> DR stride rule)
RATES = (1, 4, 8, 12)
NT = 2 + 9 * len(RATES)       # 38 tap matrices: pool, 1x1, 4 branches x 9
EPS = 1e-5
N_CORES = 8
# pass-1 spatial tiles: runs of full padded rows, 5 rows (440 cols) per tile
ROWTILES = [(5 * t, 5) for t in range(12)] + [(60, 4)]
NRT = len(ROWTILES)           # 13
# pass-2 tiles: 8 output rows each
RT2 = 8
NT2 = H // RT2                # 8


def build(n_cores: int = N_CORES):
    nc = bacc.Bacc(
        "TRN2",
        target_bir_lowering=False,
        debug=False,
        enable_asserts=False,
        num_devices=n_cores,
    )
    xs = nc.dram_tensor("xs", [CIN, H, W], F32, kind="ExternalInput")
    wt = nc.dram_tensor("wt", [P, NT * 2, 2, P], FP8, kind="ExternalInput")
    coef = nc.dram_tensor("coef", [P, 2, 12], F32, kind="ExternalInput")
    out = nc.dram_tensor("out", [COUT, H, W], F32, kind="ExternalOutput")

    with tile.TileContext(nc) as tc, ExitStack() as ctx:
        const = ctx.enter_context(tc.tile_pool(name="const", bufs=1))
        xload = ctx.enter_context(tc.tile_pool(name="xload", bufs=2))
        ppool = ctx.enter_context(
            tc.tile_pool(name="ppool", bufs=6, space=bass.MemorySpace.PSUM))
        psmall = ctx.enter_context(
            tc.tile_pool(name="psmall", bufs=2, space=bass.MemorySpace.PSUM))
        ybuf = ctx.enter_context(tc.tile_pool(name="ybuf", bufs=1))
        stat = ctx.enter_context(tc.tile_pool(name="stat", bufs=1))
        tmp = ctx.enter_context(tc.tile_pool(name="tmp", bufs=4))
        sqp = ctx.enter_context(tc.tile_pool(name="sqp", bufs=4))
        sbout = ctx.enter_context(tc.tile_pool(name="sbout", bufs=4))
        dram = ctx.enter_context(
            tc.tile_pool(name="dram", bufs=1, space=bass.MemorySpace.DRAM))

        # ---- weights + bn coefficient load (separate DMA queues: startup
        # latency is gated by x-load -> sign, so keep wt off that queue)
        lhsT = const.tile([P, NT * 2, 2, P], FP8, tag="lhsT")
        nc.scalar.dma_start(lhsT[:], wt.ap())
        coef_sb = const.tile([P, 2, 12], F32, tag="coef")
        nc.sync.dma_start(coef_sb[:], coef.ap())
        epsb = stat.tile([P, 1], F32, tag="epsb")
        nc.vector.memset(epsb[:], EPS)

        def wdr(blk):
            return lhsT[:, blk]          # [P, 2, P] fp8, k-interleaved

        # ---- x -> padded k-interleaved sign image (fp8) + pooled sign
        sxp = const.tile([P, 2, ILEN], FP8, tag="sxp")
        for i in range(2):
            # zero only pad/guard areas (interior is overwritten by Sign):
            # front guard + top pad rows; bottom pad rows + back guard; and the
            # 2*PAD-wide seams (right pad of row r | left pad of row r+1).
            nc.gpsimd.memset(sxp[:, i, 0:GUARD + PAD * PW], 0.0)
            nc.gpsimd.memset(sxp[:, i, GUARD + (PAD + H) * PW:ILEN], 0.0)
            off0 = GUARD + PAD * PW - PAD
            seams = sxp[:, i, off0:off0 + (H + 1) * PW].rearrange(
                "p (r c) -> p r c", c=PW)[:, :, 0:2 * PAD]
            nc.gpsimd.memset(seams, 0.0)
        spool = const.tile([P, 2, 16], FP8, tag="spool")  # 16-wide: DR step rule
        for kc in range(2):
            xsb = xload.tile([P, H, W], F32, tag="xsb")
            nc.sync.dma_start(xsb[:], xs.ap()[kc * P:(kc + 1) * P])
            xsum = xload.tile([P, 1], F32, tag="xsum")
            nc.vector.reduce_sum(xsum[:], xsb[:], axis=AX.XY)
            nc.scalar.activation(spool[:, kc, 0:1], xsum[:], AF.Sign)
            interior = sxp[:, kc, GUARD:GUARD + PH * PW].rearrange(
                "p (r c) -> p r c", c=PW)[:, PAD:PAD + H, PAD:PAD + W]
            nc.scalar.activation(interior, xsb[:], AF.Sign)

        # s accumulator (filled branch by branch)
        s_all = const.tile([P, 2, HW], F32, tag="s_all")

        # per-branch coefficient state
        s0 = {}
        for mc in range(2):
            s0[mc] = stat.tile([P, 1], F32, tag=f"s0_{mc}", name=f"s0_{mc}")

        def allreduce_stats(key, stats_j, width):
            st_in = dram.tile([P, width], F32, tag=f"st_in{key}",
                              name=f"st_in{key}")
            st_out = dram.tile([P, width], F32, tag=f"st_out{key}",
                               name=f"st_out{key}",
                               addr_space="Shared" if n_cores > 4 else "Local")
            nc.sync.dma_start(st_in[:], stats_j)
            nc.gpsimd.collective_compute(
                "AllReduce", ALU.add,
                replica_groups=[list(range(n_cores))],
                ins=[st_in[:].opt()], outs=[st_out[:].opt()],
            )
            ar = stat.tile([P, width], F32, tag=f"stats_ar{key}",
                           name=f"stats_ar{key}")
            nc.sync.dma_start(ar[:], st_out[:])
            return ar

        def coef_j(j, mc, ar2):
            """a, d from allreduced [sum, sumsq] ([P, 2] AP); returns (a_, d_)."""
            cntj = float(n_cores) if j == 0 else float(HW * n_cores)
            mu = tmp.tile([P, 1], F32, tag="mu")
            nc.vector.tensor_scalar(mu[:], ar2[:, 0:1], 1.0 / cntj, None,
                                    op0=ALU.mult)
            ex2 = tmp.tile([P, 1], F32, tag="ex2")
            nc.vector.tensor_scalar(ex2[:], ar2[:, 1:2], 1.0 / cntj, None,
                                    op0=ALU.mult)
            var = tmp.tile([P, 1], F32, tag="var")
            nc.vector.tensor_tensor(var[:], mu[:], mu[:], op=ALU.mult)
            nc.vector.tensor_tensor(var[:], ex2[:], var[:], op=ALU.subtract)
            std = tmp.tile([P, 1], F32, tag="std")
            nc.scalar.activation(std[:], var[:], AF.Sqrt, bias=epsb[:])
            inv = tmp.tile([P, 1], F32, tag="inv")
            nc.vector.reciprocal(inv[:], std[:])
            a_ = stat.tile([P, 1], F32, tag=f"a{j}_{mc}")
            nc.vector.tensor_tensor(a_[:], coef_sb[:, mc, 2 * j:2 * j + 1],
                                    inv[:], op=ALU.mult)
            d_ = tmp.tile([P, 1], F32, tag="d")
            nc.vector.tensor_tensor(d_[:], mu[:], a_[:], op=ALU.mult)
            nc.vector.tensor_tensor(d_[:], coef_sb[:, mc, 2 * j + 1:2 * j + 2],
                                    d_[:], op=ALU.subtract)
            return a_, d_

        # ---- pool branch (j=0): y_pool = sign(W_pool) @ sign(mean(x))
        stats_0 = stat.tile([P, 2, 2], F32, tag="stats_0")
        ypool = []
        for mc in range(2):
            yp = psmall.tile([P, 1], F32, tag="yp")
            nc.tensor.matmul(yp[:], wdr(0 * 2 + mc), spool[:, :, 0:1],
                             start=True, stop=True, perf_mode=DR)
            ys = stat.tile([P, 1], F32, tag=f"ypool{mc}")
            nc.scalar.activation(ys[:], yp[:], AF.Copy)
            nc.vector.tensor_copy(stats_0[:, mc, 0:1], ys[:])
            nc.vector.tensor_tensor(stats_0[:, mc, 1:2], ys[:], ys[:], op=ALU.mult)
            ypool.append(ys)
        ar0 = allreduce_stats("p", stats_0[:], 4)
        for mc in range(2):
            a_, d_ = coef_j(0, mc, ar0[:, 2 * mc:2 * mc + 2])
            nc.vector.tensor_tensor(s0[mc][:], a_[:], ypool[mc][:], op=ALU.mult)
            nc.vector.tensor_tensor(s0[mc][:], s0[mc][:], d_[:], op=ALU.add)

        # ---- conv branches (j=1 the 1x1, j=2..5 the dilated 3x3s)
        # stats -> allreduce -> coef -> apply run per (branch, mc) half, so
        # the final allreduce+apply overlap the last branch's other-half
        # matmuls and only one half sits in the tail.
        branches = [(1, None)] + [(2 + 9 * i, r) for i, r in enumerate(RATES)]
        NB = len(branches)

        def compute_half(j, taps, r, mc, stats_out):
            """Matmul + drain + stats for one (branch, out-channel-half).
            Writes [sum, sumsq] into stats_out ([P,2] AP); returns y16 tile."""
            yt = ybuf.tile([P, HW], F16, tag=f"y{j}_{mc}", name=f"y{j}_{mc}")
            sum_p = stat.tile([P, NRT], F32, tag=f"sump{j}_{mc}",
                              name=f"sump{j}_{mc}")
            sq_p = stat.tile([P, NRT], F32, tag=f"sqp{j}_{mc}",
                             name=f"sqp{j}_{mc}")
            for it, (h0, nr) in enumerate(ROWTILES):
                n = nr * PW
                acc = ppool.tile([P, 5 * PW], F32, tag="acc")
                for i_mm, (tap, ky, kx) in enumerate(taps):
                    rr = 0 if r is None else r
                    pos = GUARD + (PAD + h0 + rr * (ky - 1)) * PW + rr * (kx - 1)
                    rhs = sxp[:, :, pos:pos + n]
                    nc.tensor.matmul(acc[:, 0:n], wdr(tap * 2 + mc), rhs,
                                     start=(i_mm == 0),
                                     stop=(i_mm == len(taps) - 1),
                                     perf_mode=DR)
                acc3 = acc[:, 0:n].rearrange("p (r c) -> p r c", c=PW)
                useful = acc3[:, :, PAD:PAD + W]
                ysl = yt[:, h0 * W:(h0 + nr) * W]
                nc.scalar.activation(ysl, useful, AF.Copy,
                                     accum_out=sum_p[:, it:it + 1])
                sqt = sqp.tile([P, 5 * W], F32, tag="sqt")
                if j == 1:
                    # 1x1 branch: 1 matmul/tile -> ACT is the drain
                    # bottleneck; put the square pass on (then-idle) DVE
                    nc.vector.scalar_tensor_tensor(
                        sqt[:, 0:nr * W], ysl, 1.0, ysl,
                        op0=ALU.mult, op1=ALU.mult,
                        accum_out=sq_p[:, it:it + 1])
                else:
                    nc.scalar.activation(sqt[:, 0:nr * W], ysl, AF.Square,
                                         accum_out=sq_p[:, it:it + 1])
            # stats reduce on ACT (accum trick): a stalled DVE apply
            # (waiting on a previous allreduce) must not delay the stats
            red = sqp.tile([P, NRT], F32, tag="red")
            nc.scalar.activation(red[:], sum_p[:], AF.Copy,
                                 accum_out=stats_out[:, 0:1])
            nc.scalar.activation(red[:], sq_p[:], AF.Copy,
                                 accum_out=stats_out[:, 1:2])
            return yt

        def apply_half(j, mc, yt, ar2):
            """coef + BN/htanh apply for one half (DVE, overlaps later MMs)."""
            a_, d_ = coef_j(j, mc, ar2)
            inva = tmp.tile([P, 1], F32, tag="inva")
            nc.vector.reciprocal(inva[:], a_[:])
            lo = stat.tile([P, 1], F32, tag=f"lo{j}_{mc}", name=f"lo{j}_{mc}")
            nc.vector.tensor_scalar(lo[:], d_[:], -1.0, -1.0,
                                    op0=ALU.mult, op1=ALU.add)
            nc.vector.tensor_tensor(lo[:], lo[:], inva[:], op=ALU.mult)
            hi = stat.tile([P, 1], F32, tag=f"hi{j}_{mc}", name=f"hi{j}_{mc}")
            nc.vector.tensor_scalar(hi[:], d_[:], -1.0, 1.0,
                                    op0=ALU.mult, op1=ALU.add)
            nc.vector.tensor_tensor(hi[:], hi[:], inva[:], op=ALU.mult)
            nc.vector.tensor_tensor(s0[mc][:], s0[mc][:], d_[:], op=ALU.add)
            for t in range(NT2):
                ssl = s_all[:, mc, t * 512:(t + 1) * 512]
                u = sbout.tile([P, 512], F32, tag="u")
                nc.vector.tensor_scalar(u[:], yt[:, t * 512:(t + 1) * 512],
                                        lo[:], hi[:], op0=ALU.max, op1=ALU.min)
                if j == 1:
                    nc.vector.tensor_scalar(ssl, u[:], a_[:], None, op0=ALU.mult)
                else:
                    nc.vector.scalar_tensor_tensor(ssl, u[:], a_[:], ssl,
                                                   op0=ALU.mult, op1=ALU.add)

        def final_half(mc):
            """Add s0 and store; on GpSimd so it overlaps the DVE applies."""
            for t in range(NT2):
                sf = sbout.tile([P, RT2, W], F32, tag="sf")
                nc.gpsimd.tensor_scalar(
                    sf[:], s_all[:, mc, t * 512:(t + 1) * 512],
                    s0[mc][:], None, op0=ALU.add)
                nc.sync.dma_start(
                    out.ap()[mc * P:(mc + 1) * P, t * RT2:(t + 1) * RT2, :],
                    sf[:])

        for j, (tap0, r) in enumerate(branches, start=1):
            if r is None:
                taps = [(tap0, 1, 1)]
            else:
                taps = [(tap0 + 3 * ky + kx, ky, kx)
                        for ky in range(3) for kx in range(3)]
            if j < NB:
                # one allreduce for both halves of this branch
                st2 = stat.tile([P, 4], F32, tag=f"st2_{j}", name=f"st2_{j}")
                ys = [compute_half(j, taps, r, mc, st2[:, 2 * mc:2 * mc + 2])
                      for mc in range(2)]
                arj = allreduce_stats(f"{j}", st2[:], 4)
                for mc in range(2):
                    apply_half(j, mc, ys[mc], arj[:, 2 * mc:2 * mc + 2])
            else:
                # last branch: allreduce per half so the first half's
                # allreduce+apply+store overlap the second half's matmuls
                for mc in range(2):
                    st1 = stat.tile([P, 2], F32, tag=f"st1_{mc}",
                                    name=f"st1_{mc}")
                    yt = compute_half(j, taps, r, mc, st1[:])
                    arh = allreduce_stats(f"{j}_{mc}", st1[:], 2)
                    apply_half(j, mc, yt, arh[:])
                    final_half(mc)

    nc.compile()
    return nc


def pack_weights(w_pool, w1, w3):
    """Host filter transform: sign -> DoubleRow k-interleave, fp8.

    wt[k, t*2+mc, i, m] = sign(W_t[mc*128+m, i*128+k]); block (t*2+mc) is the
    stationary [2, 128] operand for logical tap t / out-channel chunk mc.
    """
    mats = [np.sign(np.asarray(w_pool, np.float32).reshape(COUT, CIN)),
            np.sign(np.asarray(w1, np.float32).reshape(COUT, CIN))]
    w3 = np.asarray(w3, np.float32)
    for i in range(len(RATES)):
        for ky in range(3):
            for kx in range(3):
                mats.append(np.sign(w3[i, :, :, ky, kx]))
    wt = np.zeros((P, NT * 2, 2, P), np.float32)  # [k, blk, i, m]
    for t, m in enumerate(mats):
        for mc in range(2):
            for i in range(2):
                blk = m[mc * P:(mc + 1) * P, i * P:(i + 1) * P]   # [m, k]
                wt[:, t * 2 + mc, i, :] = blk.T
    return wt.astype(mybir.dt.np(FP8))


def pack_coef(g_pool, b_pool, g1, b1, g3, b3):
    gs = [g_pool, g1] + [g3[i] for i in range(len(RATES))]
    bs = [b_pool, b1] + [b3[i] for i in range(len(RATES))]
    coef = np.zeros((P, 2, 12), np.float32)
    for j in range(6):
        g = np.asarray(gs[j], np.float32)
        b = np.asarray(bs[j], np.float32)
        for mc in range(2):
            coef[:, mc, 2 * j] = g[mc * P:(mc + 1) * P]
            coef[:, mc, 2 * j + 1] = b[mc * P:(mc + 1) * P]
    return coef


_NC = None


def _get_nc():
    global _NC
    if _NC is None:
        _NC = build(N_CORES)
    return _NC


def make_in_maps(x, w_pool, g_pool, b_pool, w1, g1, b1, w3, g3, b3):
    x = np.asarray(x, np.float32)
    wt = pack_weights(w_pool, w1, w3)
    coef = pack_coef(g_pool, b_pool, g1, b1, g3, b3)
    return [
        {"xs": np.ascontiguousarray(x[c]), "wt": wt, "coef": coef}
        for c in range(x.shape[0])
    ]


def kernel(x, w_pool, g_pool, b_pool, w1, g1, b1, w3, g3, b3):
    nc = _get_nc()
    in_maps = make_in_maps(x, w_pool, g_pool, b_pool, w1, g1, b1, w3, g3, b3)
    res = run_bass_kernel_spmd(nc, in_maps, core_ids=list(range(N_CORES)))
    return np.stack([res.results[c]["out"] for c in range(N_CORES)], axis=0)


# revision 36
# speedup vs baseline: 1.2826x; 1.2826x over previous
"""BinASPP Trainium2 kernel (Bass/Tile), SPMD over 8 NeuronCores.

Strategy
--------
Data-parallel over batch: N=8 images -> 1 image per core.  binarize() forward
== sign(), so every conv is a matmul over {-1,0,+1} values: exact in fp8e4,
with exact integer accumulation in fp32 PSUM.  A dilated 3x3 conv is 9
shifted 1x1 convs (taps) over a zero-padded sign image resident in SBUF.

PE efficiency:
 - the padded sign image is stored k-interleaved [128, 2, pad_image] fp8 so a
   single DoubleRow matmul contracts all K=256 input channels at once;
 - moving-operand tiles are contiguous runs of 5 *padded* rows (N=440): a
   strided multi-dim rhs AP costs ~50 PE cycles per row break (measured
   381ns vs 213ns theoretical for [8x64] tiles), so we stream whole padded
   rows and discard the 24 pad columns when reading PSUM back.

Batch-norm is training-mode (batch statistics), so per-channel [sum, sumsq]
partials are AllReduced across the 8 cores.  One collective is issued per
(branch, out-channel-half), so each half's BN apply (s += a_c * clip(y, lo_c,
hi_c), lo/hi = (-/+1 - d_c)/a_c, d_c offsets deferred to a final += s0 pass)
runs on DVE underneath the remaining matmul stream; only the very last
half's allreduce+apply sits in the tail.  Branch sums ride free on the
PSUM->SBUF fp16 copy (ACT accum_out); sumsq is one Square pass per tile
(ACT, or DVE for the 1x1 branch whose PSUM drain is ACT-bound).  The stats
reductions stay off the in-order DVE queue (a stalled apply waiting on a
previous allreduce must not delay the next stats).  Branch outputs y (even
integers, |y| <= 2304) are staged in SBUF as fp16 (exact), so the apply pass
needs no recompute and no DRAM round trip.
"""

import numpy as np
import ml_dtypes
from contextlib import ExitStack

import concourse.bass as bass
import concourse.bacc as bacc
import concourse.mybir as mybir
import concourse.tile as tile
from concourse.bass_utils import run_bass_kernel_spmd

AF = mybir.ActivationFunctionType
ALU = mybir.AluOpType
AX = mybir.AxisListType
F32 = mybir.dt.float32
BF16 = mybir.dt.bfloat16
F16 = mybir.dt.float16
FP8 = mybir.dt.float8e4
DR = mybir.MatmulPerfMode.DoubleRow

P = 128
CIN = 256
COUT = 256
H = W = 64
HW = H * W
PAD = 12                      # max dilation rate
PH = PW = H + 2 * PAD         # 88
GUARD = 16                    # fp8 guard elements before/after each image
ILEN = GUARD + PH * PW + GUARD  # 7776 (multiple of 16 -> DR stride rule)
RATES = (1, 4, 8, 12)
NT = 2 + 9 * len(RATES)       # 38 tap matrices: pool, 1x1, 4 branches x 9
EPS = 1e-5
N_CORES = 8
# pass-1 spatial tiles: runs of full padded rows, 5 rows (440 cols) per tile
ROWTILES = [(5 * t, 5) for t in range(12)] + [(60, 4)]
NRT = len(ROWTILES)           # 13
# pass-2 tiles: 8 output rows each
RT2 = 8
NT2 = H // RT2                # 8


def build(n_cores: int = N_CORES):
    nc = bacc.Bacc(
        "TRN2",
        target_bir_lowering=False,
        debug=False,
        enable_asserts=False,
        num_devices=n_cores,
    )
    xs = nc.dram_tensor("xs", [CIN, H, W], F32, kind="ExternalInput")
    wt = nc.dram_tensor("wt", [P, NT * 2, 2, P], FP8, kind="ExternalInput")
    coef = nc.dram_tensor("coef", [P, 2, 12], F32, kind="ExternalInput")
    out = nc.dram_tensor("out", [COUT, H, W], F32, kind="ExternalOutput")

    with tile.TileContext(nc) as tc, ExitStack() as ctx:
        const = ctx.enter_context(tc.tile_pool(name="const", bufs=1))
        xload = ctx.enter_context(tc.tile_pool(name="xload", bufs=2))
        ppool = ctx.enter_context(
            tc.tile_pool(name="ppool", bufs=6, space=bass.MemorySpace.PSUM))
        psmall = ctx.enter_context(
            tc.tile_pool(name="psmall", bufs=2, space=bass.MemorySpace.PSUM))
        ybuf = ctx.enter_context(tc.tile_pool(name="ybuf", bufs=1))
        stat = ctx.enter_context(tc.tile_pool(name="stat", bufs=1))
        tmp = ctx.enter_context(tc.tile_pool(name="tmp", bufs=4))
        sqp = ctx.enter_context(tc.tile_pool(name="sqp", bufs=4))
        sbout = ctx.enter_context(tc.tile_pool(name="sbout", bufs=4))
        dram = ctx.enter_context(
            tc.tile_pool(name="dram", bufs=1, space=bass.MemorySpace.DRAM))

        # ---- weights + bn coefficient load (separate DMA queues: startup
        # latency is gated by x-load -> sign, so keep wt off that queue)
        lhsT = const.tile([P, NT * 2, 2, P], FP8, tag="lhsT")
        nc.scalar.dma_start(lhsT[:], wt.ap())
        coef_sb = const.tile([P, 2, 12], F32, tag="coef")
        nc.sync.dma_start(coef_sb[:], coef.ap())
        epsb = stat.tile([P, 1], F32, tag="epsb")
        nc.vector.memset(epsb[:], EPS)

        def wdr(blk):
            return lhsT[:, blk]          # [P, 2, P] fp8, k-interleaved

        # ---- x -> padded k-interleaved sign image (fp8) + pooled sign
        sxp = const.tile([P, 2, ILEN], FP8, tag="sxp")
        for i in range(2):
            # zero only pad/guard areas (interior is overwritten by Sign):
            # front guard + top pad rows; bottom pad rows + back guard; and the
            # 2*PAD-wide seams (right pad of row r | left pad of row r+1).
            nc.gpsimd.memset(sxp[:, i, 0:GUARD + PAD * PW], 0.0)
            nc.gpsimd.memset(sxp[:, i, GUARD + (PAD + H) * PW:ILEN], 0.0)
            off0 = GUARD + PAD * PW - PAD
            seams = sxp[:, i, off0:off0 + (H + 1) * PW].rearrange(
                "p (r c) -> p r c", c=PW)[:, :, 0:2 * PAD]
            nc.gpsimd.memset(seams, 0.0)
        spool = const.tile([P, 2, 16], FP8, tag="spool")  # 16-wide: DR step rule
        for kc in range(2):
            xsb = xload.tile([P, H, W], F32, tag="xsb")
            nc.sync.dma_start(xsb[:], xs.ap()[kc * P:(kc + 1) * P])
            xsum = xload.tile([P, 1], F32, tag="xsum")
            nc.vector.reduce_sum(xsum[:], xsb[:], axis=AX.XY)
            nc.scalar.activation(spool[:, kc, 0:1], xsum[:], AF.Sign)
            interior = sxp[:, kc, GUARD:GUARD + PH * PW].rearrange(
                "p (r c) -> p r c", c=PW)[:, PAD:PAD + H, PAD:PAD + W]
            nc.scalar.activation(interior, xsb[:], AF.Sign)

        # s accumulator (filled branch by branch)
        s_all = const.tile([P, 2, HW], F32, tag="s_all")

        # per-branch coefficient state
        s0 = {}
        for mc in range(2):
            s0[mc] = stat.tile([P, 1], F32, tag=f"s0_{mc}", name=f"s0_{mc}")

        def allreduce_stats(key, stats_j, width):
            st_in = dram.tile([P, width], F32, tag=f"st_in{key}",
                              name=f"st_in{key}")
            st_out = dram.tile([P, width], F32, tag=f"st_out{key}",
                               name=f"st_out{key}",
                               addr_space="Shared" if n_cores > 4 else "Local")
            nc.sync.dma_start(st_in[:], stats_j)
            nc.gpsimd.collective_compute(
                "AllReduce", ALU.add,
                replica_groups=[list(range(n_cores))],
                ins=[st_in[:].opt()], outs=[st_out[:].opt()],
            )
            ar = stat.tile([P, width], F32, tag=f"stats_ar{key}",
                           name=f"stats_ar{key}")
            nc.sync.dma_start(ar[:], st_out[:])
            return ar

        def coef_j(j, mc, ar2):
            """a, d from allreduced [sum, sumsq] ([P, 2] AP); returns (a_, d_)."""
            cntj = float(n_cores) if j == 0 else float(HW * n_cores)
            mu = tmp.tile([P, 1], F32, tag="mu")
            nc.vector.tensor_scalar(mu[:], ar2[:, 0:1], 1.0 / cntj, None,
                                    op0=ALU.mult)
            ex2 = tmp.tile([P, 1], F32, tag="ex2")
            nc.vector.tensor_scalar(ex2[:], ar2[:, 1:2], 1.0 / cntj, None,
                                    op0=ALU.mult)
            var = tmp.tile([P, 1], F32, tag="var")
            nc.vector.tensor_tensor(var[:], mu[:], mu[:], op=ALU.mult)
            nc.vector.tensor_tensor(var[:], ex2[:], var[:], op=ALU.subtract)
            std = tmp.tile([P, 1], F32, tag="std")
            nc.scalar.activation(std[:], var[:], AF.Sqrt, bias=epsb[:])
            inv = tmp.tile([P, 1], F32, tag="inv")
            nc.vector.reciprocal(inv[:], std[:])
            a_ = stat.tile([P, 1], F32, tag=f"a{j}_{mc}")
            nc.vector.tensor_tensor(a_[:], coef_sb[:, mc, 2 * j:2 * j + 1],
                                    inv[:], op=ALU.mult)
            d_ = tmp.tile([P, 1], F32, tag="d")
            nc.vector.tensor_tensor(d_[:], mu[:], a_[:], op=ALU.mult)
            nc.vector.tensor_tensor(d_[:], coef_sb[:, mc, 2 * j + 1:2 * j + 2],
                                    d_[:], op=ALU.subtract)
            return a_, d_

        # ---- pool branch (j=0): y_pool = sign(W_pool) @ sign(mean(x))
        stats_0 = stat.tile([P, 2, 2], F32, tag="stats_0")
        ypool = []
        for mc in range(2):
            yp = psmall.tile([P, 1], F32, tag="yp")
            nc.tensor.matmul(yp[:], wdr(0 * 2 + mc), spool[:, :, 0:1],
                             start=True, stop=True, perf_mode=DR)
            ys = stat.tile([P, 1], F32, tag=f"ypool{mc}")
            nc.scalar.activation(ys[:], yp[:], AF.Copy)
            nc.vector.tensor_copy(stats_0[:, mc, 0:1], ys[:])
            nc.vector.tensor_tensor(stats_0[:, mc, 1:2], ys[:], ys[:], op=ALU.mult)
            ypool.append(ys)
        ar0 = allreduce_stats("p", stats_0[:], 4)
        for mc in range(2):
            a_, d_ = coef_j(0, mc, ar0[:, 2 * mc:2 * mc + 2])
            nc.vector.tensor_tensor(s0[mc][:], a_[:], ypool[mc][:], op=ALU.mult)
            nc.vector.tensor_tensor(s0[mc][:], s0[mc][:], d_[:], op=ALU.add)

        # ---- conv branches (j=1 the 1x1, j=2..5 the dilated 3x3s)
        # stats -> allreduce -> coef -> apply run per (branch, mc) half, so
        # the final allreduce+apply overlap the last branch's other-half
        # matmuls and only one half sits in the tail.
        branches = [(1, None)] + [(2 + 9 * i, r) for i, r in enumerate(RATES)]
        NB = len(branches)

        def compute_half(j, taps, r, mc, stats_out):
            """Matmul + drain + stats for one (branch, out-channel-half).
            Writes [sum, sumsq] into stats_out ([P,2] AP); returns y16 tile."""
            yt = ybuf.tile([P, HW], F16, tag=f"y{j}_{mc}", name=f"y{j}_{mc}")
            sum_p = stat.tile([P, NRT], F32, tag=f"sump{j}_{mc}",
                              name=f"sump{j}_{mc}")
            sq_p = stat.tile([P, NRT], F32, tag=f"sqp{j}_{mc}",
                             name=f"sqp{j}_{mc}")
            for it, (h0, nr) in enumerate(ROWTILES):
                n = nr * PW
                acc = ppool.tile([P, 5 * PW], F32, tag="acc")
                for i_mm, (tap, ky, kx) in enumerate(taps):
                    rr = 0 if r is None else r
                    pos = GUARD + (PAD + h0 + rr * (ky - 1)) * PW + rr * (kx - 1)
                    rhs = sxp[:, :, pos:pos + n]
                    nc.tensor.matmul(acc[:, 0:n], wdr(tap * 2 + mc), rhs,
                                     start=(i_mm == 0),
                                     stop=(i_mm == len(taps) - 1),
                                     perf_mode=DR)
                acc3 = acc[:, 0:n].rearrange("p (r c) -> p r c", c=PW)
                useful = acc3[:, :, PAD:PAD + W]
                ysl = yt[:, h0 * W:(h0 + nr) * W]
                nc.scalar.activation(ysl, useful, AF.Copy,
                                     accum_out=sum_p[:, it:it + 1])
                sqt = sqp.tile([P, 5 * W], F32, tag="sqt")
                if j == 1:
                    # 1x1 branch: 1 matmul/tile -> ACT is the drain
                    # bottleneck; put the square pass on (then-idle) DVE
                    nc.vector.scalar_tensor_tensor(
                        sqt[:, 0:nr * W], ysl, 1.0, ysl,
                        op0=ALU.mult, op1=ALU.mult,
                        accum_out=sq_p[:, it:it + 1])
                else:
                    nc.scalar.activation(sqt[:, 0:nr * W], ysl, AF.Square,
                                         accum_out=sq_p[:, it:it + 1])
            # stats reduce on ACT (accum trick): a stalled DVE apply
            # (waiting on a previous allreduce) must not delay the stats
            red = sqp.tile([P, NRT], F32, tag="red")
            nc.scalar.activation(red[:], sum_p[:], AF.Copy,
                                 accum_out=stats_out[:, 0:1])
            nc.scalar.activation(red[:], sq_p[:], AF.Copy,
                                 accum_out=stats_out[:, 1:2])
            return yt

        def apply_half(j, mc, yt, ar2):
            """coef + BN/htanh apply for one half (DVE, overlaps later MMs)."""
            a_, d_ = coef_j(j, mc, ar2)
            inva = tmp.tile([P, 1], F32, tag="inva")
            nc.vector.reciprocal(inva[:], a_[:])
            lo = stat.tile([P, 1], F32, tag=f"lo{j}_{mc}", name=f"lo{j}_{mc}")
            nc.vector.tensor_scalar(lo[:], d_[:], -1.0, -1.0,
                                    op0=ALU.mult, op1=ALU.add)
            nc.vector.tensor_tensor(lo[:], lo[:], inva[:], op=ALU.mult)
            hi = stat.tile([P, 1], F32, tag=f"hi{j}_{mc}", name=f"hi{j}_{mc}")
            nc.vector.tensor_scalar(hi[:], d_[:], -1.0, 1.0,
                                    op0=ALU.mult, op1=ALU.add)
            nc.vector.tensor_tensor(hi[:], hi[:], inva[:], op=ALU.mult)
            nc.vector.tensor_tensor(s0[mc][:], s0[mc][:], d_[:], op=ALU.add)
            for t in range(NT2):
                ssl = s_all[:, mc, t * 512:(t + 1) * 512]
                u = sbout.tile([P, 512], F32, tag="u")
                nc.vector.tensor_scalar(u[:], yt[:, t * 512:(t + 1) * 512],
                                        lo[:], hi[:], op0=ALU.max, op1=ALU.min)
                if j == 1:
                    nc.vector.tensor_scalar(ssl, u[:], a_[:], None, op0=ALU.mult)
                else:
                    nc.vector.scalar_tensor_tensor(ssl, u[:], a_[:], ssl,
                                                   op0=ALU.mult, op1=ALU.add)

        def final_half(mc):
            """Add s0 and store (DVE; gpsimd elementwise measured ~20x slower)."""
            for t in range(NT2):
                sf = sbout.tile([P, RT2, W], F32, tag="sf")
                nc.vector.tensor_scalar(
                    sf[:], s_all[:, mc, t * 512:(t + 1) * 512],
                    s0[mc][:], None, op0=ALU.add)
                nc.sync.dma_start(
                    out.ap()[mc * P:(mc + 1) * P, t * RT2:(t + 1) * RT2, :],
                    sf[:])

        for j, (tap0, r) in enumerate(branches, start=1):
            if r is None:
                taps = [(tap0, 1, 1)]
            else:
                taps = [(tap0 + 3 * ky + kx, ky, kx)
                        for ky in range(3) for kx in range(3)]
            if j < NB:
                # one allreduce for both halves of this branch
                st2 = stat.tile([P, 4], F32, tag=f"st2_{j}", name=f"st2_{j}")
                ys = [compute_half(j, taps, r, mc, st2[:, 2 * mc:2 * mc + 2])
                      for mc in range(2)]
                arj = allreduce_stats(f"{j}", st2[:], 4)
                for mc in range(2):
                    apply_half(j, mc, ys[mc], arj[:, 2 * mc:2 * mc + 2])
            else:
                # last branch: allreduce per half so the first half's
                # allreduce+apply+store overlap the second half's matmuls
                for mc in range(2):
                    st1 = stat.tile([P, 2], F32, tag=f"st1_{mc}",
                                    name=f"st1_{mc}")
                    yt = compute_half(j, taps, r, mc, st1[:])
                    arh = allreduce_stats(f"{j}_{mc}", st1[:], 2)
                    apply_half(j, mc, yt, arh[:])
                    final_half(mc)

    nc.compile()
    return nc


def pack_weights(w_pool, w1, w3):
    """Host filter transform: sign -> DoubleRow k-interleave, fp8.

    wt[k, t*2+mc, i, m] = sign(W_t[mc*128+m, i*128+k]); block (t*2+mc) is the
    stationary [2, 128] operand for logical tap t / out-channel chunk mc.
    """
    mats = [np.sign(np.asarray(w_pool, np.float32).reshape(COUT, CIN)),
            np.sign(np.asarray(w1, np.float32).reshape(COUT, CIN))]
    w3 = np.asarray(w3, np.float32)
    for i in range(len(RATES)):
        for ky in range(3):
            for kx in range(3):
                mats.append(np.sign(w3[i, :, :, ky, kx]))
    wt = np.zeros((P, NT * 2, 2, P), np.float32)  # [k, blk, i, m]
    for t, m in enumerate(mats):
        for mc in range(2):
            for i in range(2):
                blk = m[mc * P:(mc + 1) * P, i * P:(i + 1) * P]   # [m, k]
                wt[:, t * 2 + mc, i, :] = blk.T
    return wt.astype(mybir.dt.np(FP8))


def pack_coef(g_pool, b_pool, g1, b1, g3, b3):
    gs = [g_pool, g1] + [g3[i] for i in range(len(RATES))]
    bs = [b_pool, b1] + [b3[i] for i in range(len(RATES))]
    coef = np.zeros((P, 2, 12), np.float32)
    for j in range(6):
        g = np.asarray(gs[j], np.float32)
        b = np.asarray(bs[j], np.float32)
        for mc in range(2):
            coef[:, mc, 2 * j] = g[mc * P:(mc + 1) * P]
            coef[:, mc, 2 * j + 1] = b[mc * P:(mc + 1) * P]
    return coef


_NC = None


def _get_nc():
    global _NC
    if _NC is None:
        _NC = build(N_CORES)
    return _NC


def make_in_maps(x, w_pool, g_pool, b_pool, w1, g1, b1, w3, g3, b3):
    x = np.asarray(x, np.float32)
    wt = pack_weights(w_pool, w1, w3)
    coef = pack_coef(g_pool, b_pool, g1, b1, g3, b3)
    return [
        {"xs": np.ascontiguousarray(x[c]), "wt": wt, "coef": coef}
        for c in range(x.shape[0])
    ]


def kernel(x, w_pool, g_pool, b_pool, w1, g1, b1, w3, g3, b3):
    nc = _get_nc()
    in_maps = make_in_maps(x, w_pool, g_pool, b_pool, w1, g1, b1, w3, g3, b3)
    res = run_bass_kernel_spmd(nc, in_maps, core_ids=list(range(N_CORES)))
    return np.stack([res.results[c]["out"] for c in range(N_CORES)], axis=0)


# revision 37
# speedup vs baseline: 1.3101x; 1.0214x over previous
"""BinASPP Trainium2 kernel (Bass/Tile), SPMD over 8 NeuronCores.

Strategy
--------
Data-parallel over batch: N=8 images -> 1 image per core.  binarize() forward
== sign(), so every conv is a matmul over {-1,0,+1} values: exact in fp8e4,
with exact integer accumulation in fp32 PSUM.  A dilated 3x3 conv is 9
shifted 1x1 convs (taps) over a zero-padded sign image resident in SBUF.

PE efficiency:
 - the padded sign image is stored k-interleaved [128, 2, pad_image] fp8 so a
   single DoubleRow matmul contracts all K=256 input channels at once;
 - moving-operand tiles are contiguous runs of 5 *padded* rows (N=440): a
   strided multi-dim rhs AP costs ~50 PE cycles per row break (measured
   381ns vs 213ns theoretical for [8x64] tiles), so we stream whole padded
   rows and discard the 24 pad columns when reading PSUM back.

Batch-norm is training-mode (batch statistics), so per-branch per-channel
[sum, sumsq] partials are AllReduced across the 8 cores.  The collectives are
issued per branch, so each branch's BN apply (s += a_c * clip(y, lo_c, hi_c),
lo/hi = (-/+1 - d_c)/a_c, d_c offsets deferred to a final += s0 pass) runs on
DVE underneath the next branches' matmul stream; only the last branch's
allreduce+apply sits in the tail.  Branch sums ride free on the PSUM->SBUF
fp16 copy (ACT accum_out); sumsq is one DVE pass per tile (accum_out).
Branch outputs y (even integers, |y| <= 2304) are staged in SBUF as fp16
(exact), so pass 2 needs no recompute and no DRAM round trip.
"""

import numpy as np
import ml_dtypes
from contextlib import ExitStack

import concourse.bass as bass
import concourse.bacc as bacc
import concourse.mybir as mybir
import concourse.tile as tile
from concourse.bass_utils import run_bass_kernel_spmd

AF = mybir.ActivationFunctionType
ALU = mybir.AluOpType
AX = mybir.AxisListType
F32 = mybir.dt.float32
BF16 = mybir.dt.bfloat16
F16 = mybir.dt.float16
FP8 = mybir.dt.float8e4
DR = mybir.MatmulPerfMode.DoubleRow

P = 128
CIN = 256
COUT = 256
H = W = 64
HW = H * W
PAD = 12                      # max dilation rate
PH = PW = H + 2 * PAD         # 88
GUARD = 16                    # fp8 guard elements before/after each image
ILEN = GUARD + PH * PW + GUARD  # 7776 (multiple of 16 -> DR stride rule)
RATES = (1, 4, 8, 12)
NT = 2 + 9 * len(RATES)       # 38 tap matrices: pool, 1x1, 4 branches x 9
EPS = 1e-5
N_CORES = 8
# pass-1 spatial tiles: runs of full padded rows, 5 rows (440 cols) per tile
ROWTILES = [(5 * t, 5) for t in range(12)] + [(60, 4)]
NRT = len(ROWTILES)           # 13
# pass-2 tiles: 8 output rows each
RT2 = 8
NT2 = H // RT2                # 8


def build(n_cores: int = N_CORES):
    nc = bacc.Bacc(
        "TRN2",
        target_bir_lowering=False,
        debug=False,
        enable_asserts=False,
        num_devices=n_cores,
    )
    xs = nc.dram_tensor("xs", [CIN, H, W], F32, kind="ExternalInput")
    wt = nc.dram_tensor("wt", [P, NT * 2, 2, P], FP8, kind="ExternalInput")
    coef = nc.dram_tensor("coef", [P, 2, 12], F32, kind="ExternalInput")
    out = nc.dram_tensor("out", [COUT, H, W], F32, kind="ExternalOutput")

    with tile.TileContext(nc) as tc, ExitStack() as ctx:
        const = ctx.enter_context(tc.tile_pool(name="const", bufs=1))
        xload = ctx.enter_context(tc.tile_pool(name="xload", bufs=2))
        ppool = ctx.enter_context(
            tc.tile_pool(name="ppool", bufs=6, space=bass.MemorySpace.PSUM))
        psmall = ctx.enter_context(
            tc.tile_pool(name="psmall", bufs=2, space=bass.MemorySpace.PSUM))
        ybuf = ctx.enter_context(tc.tile_pool(name="ybuf", bufs=1))
        stat = ctx.enter_context(tc.tile_pool(name="stat", bufs=1))
        tmp = ctx.enter_context(tc.tile_pool(name="tmp", bufs=4))
        sqp = ctx.enter_context(tc.tile_pool(name="sqp", bufs=4))
        sbout = ctx.enter_context(tc.tile_pool(name="sbout", bufs=4))
        dram = ctx.enter_context(
            tc.tile_pool(name="dram", bufs=1, space=bass.MemorySpace.DRAM))

        # ---- weights + bn coefficient load
        lhsT = const.tile([P, NT * 2, 2, P], FP8, tag="lhsT")
        nc.sync.dma_start(lhsT[:], wt.ap())
        coef_sb = const.tile([P, 2, 12], F32, tag="coef")
        nc.sync.dma_start(coef_sb[:], coef.ap())
        epsb = stat.tile([P, 1], F32, tag="epsb")
        nc.vector.memset(epsb[:], EPS)

        def wdr(blk):
            return lhsT[:, blk]          # [P, 2, P] fp8, k-interleaved

        # ---- x -> padded k-interleaved sign image (fp8) + pooled sign
        sxp = const.tile([P, 2, ILEN], FP8, tag="sxp")
        for i in range(2):
            # zero only pad/guard areas (interior is overwritten by Sign):
            # front guard + top pad rows; bottom pad rows + back guard; and the
            # 2*PAD-wide seams (right pad of row r | left pad of row r+1).
            nc.gpsimd.memset(sxp[:, i, 0:GUARD + PAD * PW], 0.0)
            nc.gpsimd.memset(sxp[:, i, GUARD + (PAD + H) * PW:ILEN], 0.0)
            off0 = GUARD + PAD * PW - PAD
            seams = sxp[:, i, off0:off0 + (H + 1) * PW].rearrange(
                "p (r c) -> p r c", c=PW)[:, :, 0:2 * PAD]
            nc.gpsimd.memset(seams, 0.0)
        spool = const.tile([P, 2, 16], FP8, tag="spool")  # 16-wide: DR step rule
        for kc in range(2):
            xsb = xload.tile([P, H, W], F32, tag="xsb")
            nc.sync.dma_start(xsb[:], xs.ap()[kc * P:(kc + 1) * P])
            xsum = xload.tile([P, 1], F32, tag="xsum")
            nc.vector.reduce_sum(xsum[:], xsb[:], axis=AX.XY)
            nc.scalar.activation(spool[:, kc, 0:1], xsum[:], AF.Sign)
            interior = sxp[:, kc, GUARD:GUARD + PH * PW].rearrange(
                "p (r c) -> p r c", c=PW)[:, PAD:PAD + H, PAD:PAD + W]
            nc.scalar.activation(interior, xsb[:], AF.Sign)

        # s accumulator (filled branch by branch)
        s_all = const.tile([P, 2, HW], F32, tag="s_all")

        # per-branch coefficient state
        s0 = {}
        for mc in range(2):
            s0[mc] = stat.tile([P, 1], F32, tag=f"s0_{mc}", name=f"s0_{mc}")

        def allreduce_stats(j, stats_j):
            st_in = dram.tile([P, 2, 2], F32, tag=f"st_in{j}")
            st_out = dram.tile([P, 2, 2], F32, tag=f"st_out{j}",
                               addr_space="Shared" if n_cores > 4 else "Local")
            nc.sync.dma_start(st_in[:], stats_j[:])
            nc.gpsimd.collective_compute(
                "AllReduce", ALU.add,
                replica_groups=[list(range(n_cores))],
                ins=[st_in[:].opt()], outs=[st_out[:].opt()],
            )
            ar = stat.tile([P, 2, 2], F32, tag=f"stats_ar{j}")
            nc.sync.dma_start(ar[:], st_out[:])
            return ar

        def coef_j(j, mc, ar):
            """a, d from allreduced [sum, sumsq]; returns (a_, d_)."""
            cntj = float(n_cores) if j == 0 else float(HW * n_cores)
            mu = tmp.tile([P, 1], F32, tag="mu")
            nc.vector.tensor_scalar(mu[:], ar[:, mc, 0:1], 1.0 / cntj, None,
                                    op0=ALU.mult)
            ex2 = tmp.tile([P, 1], F32, tag="ex2")
            nc.vector.tensor_scalar(ex2[:], ar[:, mc, 1:2], 1.0 / cntj, None,
                                    op0=ALU.mult)
            var = tmp.tile([P, 1], F32, tag="var")
            nc.vector.tensor_tensor(var[:], mu[:], mu[:], op=ALU.mult)
            nc.vector.tensor_tensor(var[:], ex2[:], var[:], op=ALU.subtract)
            std = tmp.tile([P, 1], F32, tag="std")
            nc.scalar.activation(std[:], var[:], AF.Sqrt, bias=epsb[:])
            inv = tmp.tile([P, 1], F32, tag="inv")
            nc.vector.reciprocal(inv[:], std[:])
            a_ = stat.tile([P, 1], F32, tag=f"a{j}_{mc}")
            nc.vector.tensor_tensor(a_[:], coef_sb[:, mc, 2 * j:2 * j + 1],
                                    inv[:], op=ALU.mult)
            d_ = tmp.tile([P, 1], F32, tag="d")
            nc.vector.tensor_tensor(d_[:], mu[:], a_[:], op=ALU.mult)
            nc.vector.tensor_tensor(d_[:], coef_sb[:, mc, 2 * j + 1:2 * j + 2],
                                    d_[:], op=ALU.subtract)
            return a_, d_

        # ---- pool branch (j=0): y_pool = sign(W_pool) @ sign(mean(x))
        stats_0 = stat.tile([P, 2, 2], F32, tag="stats_0")
        ypool = []
        for mc in range(2):
            yp = psmall.tile([P, 1], F32, tag="yp")
            nc.tensor.matmul(yp[:], wdr(0 * 2 + mc), spool[:, :, 0:1],
                             start=True, stop=True, perf_mode=DR)
            ys = stat.tile([P, 1], F32, tag=f"ypool{mc}")
            nc.scalar.activation(ys[:], yp[:], AF.Copy)
            nc.vector.tensor_copy(stats_0[:, mc, 0:1], ys[:])
            nc.vector.tensor_tensor(stats_0[:, mc, 1:2], ys[:], ys[:], op=ALU.mult)
            ypool.append(ys)
        ar0 = allreduce_stats(0, stats_0)
        for mc in range(2):
            a_, d_ = coef_j(0, mc, ar0)
            nc.vector.tensor_tensor(s0[mc][:], a_[:], ypool[mc][:], op=ALU.mult)
            nc.vector.tensor_tensor(s0[mc][:], s0[mc][:], d_[:], op=ALU.add)

        # ---- conv branches (j=1 the 1x1, j=2..5 the dilated 3x3s)
        branches = [(1, None)] + [(2 + 9 * i, r) for i, r in enumerate(RATES)]
        for j, (tap0, r) in enumerate(branches, start=1):
            if r is None:
                taps = [(tap0, 1, 1)]
            else:
                taps = [(tap0 + 3 * ky + kx, ky, kx)
                        for ky in range(3) for kx in range(3)]
            stats_j = stat.tile([P, 2, 2], F32, tag=f"stats_{j}")
            y16 = {}
            for mc in range(2):
                yt = ybuf.tile([P, HW], F16, tag=f"y{j}_{mc}")
                y16[mc] = yt
                sum_p = stat.tile([P, NRT], F32, tag=f"sump{j}_{mc}")
                sq_p = stat.tile([P, NRT], F32, tag=f"sqp{j}_{mc}")
                for it, (h0, nr) in enumerate(ROWTILES):
                    n = nr * PW
                    acc = ppool.tile([P, 5 * PW], F32, tag="acc")
                    for i_mm, (tap, ky, kx) in enumerate(taps):
                        rr = 0 if r is None else r
                        pos = GUARD + (PAD + h0 + rr * (ky - 1)) * PW + rr * (kx - 1)
                        rhs = sxp[:, :, pos:pos + n]
                        nc.tensor.matmul(acc[:, 0:n], wdr(tap * 2 + mc), rhs,
                                         start=(i_mm == 0),
                                         stop=(i_mm == len(taps) - 1),
                                         perf_mode=DR)
                    acc3 = acc[:, 0:n].rearrange("p (r c) -> p r c", c=PW)
                    useful = acc3[:, :, PAD:PAD + W]
                    ysl = yt[:, h0 * W:(h0 + nr) * W]
                    nc.scalar.activation(ysl, useful, AF.Copy,
                                         accum_out=sum_p[:, it:it + 1])
                    sqt = sqp.tile([P, 5 * W], F32, tag="sqt")
                    nc.scalar.activation(sqt[:, 0:nr * W], ysl, AF.Square,
                                         accum_out=sq_p[:, it:it + 1])
                # off the DVE queue: a stalled apply (waiting on a previous
                # branch's allreduce) must not delay this branch's stats
                red = sqp.tile([P, NRT], F32, tag="red")
                nc.scalar.activation(red[:], sum_p[:], AF.Copy,
                                     accum_out=stats_j[:, mc, 0:1])
                nc.scalar.activation(red[:], sq_p[:], AF.Copy,
                                     accum_out=stats_j[:, mc, 1:2])
            arj = allreduce_stats(j, stats_j)
            for mc in range(2):
                a_, d_ = coef_j(j, mc, arj)
                inva = tmp.tile([P, 1], F32, tag="inva")
                nc.vector.reciprocal(inva[:], a_[:])
                lo = stat.tile([P, 1], F32, tag=f"lo{j}_{mc}")
                nc.vector.tensor_scalar(lo[:], d_[:], -1.0, -1.0,
                                        op0=ALU.mult, op1=ALU.add)
                nc.vector.tensor_tensor(lo[:], lo[:], inva[:], op=ALU.mult)
                hi = stat.tile([P, 1], F32, tag=f"hi{j}_{mc}")
                nc.vector.tensor_scalar(hi[:], d_[:], -1.0, 1.0,
                                        op0=ALU.mult, op1=ALU.add)
                nc.vector.tensor_tensor(hi[:], hi[:], inva[:], op=ALU.mult)
                nc.vector.tensor_tensor(s0[mc][:], s0[mc][:], d_[:], op=ALU.add)
                # apply branch j on DVE (overlaps later branches' matmuls)
                for t in range(NT2):
                    ssl = s_all[:, mc, t * 512:(t + 1) * 512]
                    u = sbout.tile([P, 512], F32, tag="u")
                    nc.vector.tensor_scalar(u[:], y16[mc][:, t * 512:(t + 1) * 512],
                                            lo[:], hi[:], op0=ALU.max, op1=ALU.min)
                    if j == 1:
                        nc.vector.tensor_scalar(ssl, u[:], a_[:], None,
                                                op0=ALU.mult)
                    else:
                        nc.vector.scalar_tensor_tensor(ssl, u[:], a_[:], ssl,
                                                       op0=ALU.mult, op1=ALU.add)

        # ---- final: add s0 (pool value + all BN offsets), store
        for mc in range(2):
            for t in range(NT2):
                sf = sbout.tile([P, RT2, W], F32, tag="sf")
                nc.vector.tensor_scalar(sf[:], s_all[:, mc, t * 512:(t + 1) * 512],
                                        1.0, s0[mc][:], op0=ALU.mult, op1=ALU.add)
                nc.sync.dma_start(
                    out.ap()[mc * P:(mc + 1) * P, t * RT2:(t + 1) * RT2, :], sf[:])

    nc.compile()
    return nc


def pack_weights(w_pool, w1, w3):
    """Host filter transform: sign -> DoubleRow k-interleave, fp8.

    wt[k, t*2+mc, i, m] = sign(W_t[mc*128+m, i*128+k]); block (t*2+mc) is the
    stationary [2, 128] operand for logical tap t / out-channel chunk mc.
    """
    mats = [np.sign(np.asarray(w_pool, np.float32).reshape(COUT, CIN)),
            np.sign(np.asarray(w1, np.float32).reshape(COUT, CIN))]
    w3 = np.asarray(w3, np.float32)
    for i in range(len(RATES)):
        for ky in range(3):
            for kx in range(3):
                mats.append(np.sign(w3[i, :, :, ky, kx]))
    wt = np.zeros((P, NT * 2, 2, P), np.float32)  # [k, blk, i, m]
    for t, m in enumerate(mats):
        for mc in range(2):
            for i in range(2):
                blk = m[mc * P:(mc + 1) * P, i * P:(i + 1) * P]   # [m, k]
                wt[:, t * 2 + mc, i, :] = blk.T
    return wt.astype(mybir.dt.np(FP8))


def pack_coef(g_pool, b_pool, g1, b1, g3, b3):
    gs = [g_pool, g1] + [g3[i] for i in range(len(RATES))]
    bs = [b_pool, b1] + [b3[i] for i in range(len(RATES))]
    coef = np.zeros((P, 2, 12), np.float32)
    for j in range(6):
        g = np.asarray(gs[j], np.float32)
        b = np.asarray(bs[j], np.float32)
        for mc in range(2):
            coef[:, mc, 2 * j] = g[mc * P:(mc + 1) * P]
            coef[:, mc, 2 * j + 1] = b[mc * P:(mc + 1) * P]
    return coef


_NC = None


def _get_nc():
    global _NC
    if _NC is None:
        _NC = build(N_CORES)
    return _NC


def make_in_maps(x, w_pool, g_pool, b_pool, w1, g1, b1, w3, g3, b3):
    x = np.asarray(x, np.float32)
    wt = pack_weights(w_pool, w1, w3)
    coef = pack_coef(g_pool, b_pool, g1, b1, g3, b3)
    return [
        {"xs": np.ascontiguousarray(x[c]), "wt": wt, "coef": coef}
        for c in range(x.shape[0])
    ]


def kernel(x, w_pool, g_pool, b_pool, w1, g1, b1, w3, g3, b3):
    nc = _get_nc()
    in_maps = make_in_maps(x, w_pool, g_pool, b_pool, w1, g1, b1, w3, g3, b3)
    res = run_bass_kernel_spmd(nc, in_maps, core_ids=list(range(N_CORES)))
    return np.stack([res.results[c]["out"] for c in range(N_CORES)], axis=0)
